# revision 1
# baseline (speedup 1.0000x reference)
"""Trainium2 Bass kernel for batched pairwise-distance + group-min + mean.

Computes, for x1 [8, 2048, 1024] f32 and x2 [8, 1152, 1024] f32:
    d[b, m, n] = ||x1[b,m] - x2[b,n]||^2           [8, 2048, 1152]
    out = mean over groups-of-9 minima of d (reshape [B, -1, 9].min(-1).mean())

Strategy: data-parallel over batch B=8 across the 8 NeuronCores. Each core:
  - cast-DMAs its x1/x2 shard to bf16 in SBUF, xbar-transpose-DMAs them to
    [d, n] layout,
  - computes cross[m, n] = x1 @ x2.T via bf16 matmuls accumulated in PSUM,
    with an extra K=1 matmul appending -0.5*||x2[n]||^2 per column,
  - group-MAX of (cross - 0.5*sq2) over 9 consecutive n on the vector engine
    (min of d is sq1[m] - 2 * that max; sq1 is constant within a group),
  - accumulates per-partition sums; host combines:
        sum_d_min = 128 * sum(sq1) - 2 * sum(group_max_sums)
"""
import os
import sys

for _p in ("/opt/trn_rl_repo",):
    if os.path.isdir(_p) and _p not in sys.path:
        sys.path.append(_p)

import numpy as np

B = 8
N1, D, N2 = 2048, 1024, 1152
GROUP = 9
MT, KT = N1 // 128, D // 128          # 16 m-tiles, 8 k-chunks
NG = N2 // GROUP                       # 128 groups per m-row
# psum free-dim chunks, each within one 2 KiB psum bank
CHUNKS = ((0, 512), (512, 512), (1024, 128))

_CACHE = {}


def _build():
    """Build + compile the per-core Bass program once per process."""
    from concourse import bacc, tile, mybir

    F32 = mybir.dt.float32
    BF = mybir.dt.bfloat16
    AX = mybir.AxisListType
    AF = mybir.ActivationFunctionType

    nc = bacc.Bacc("TRN2", target_bir_lowering=False, debug=False, num_devices=B,
                   dynamic_dma_scratch_size=65536)
    x1_d = nc.dram_tensor("x1", [N1, D], F32, kind="ExternalInput")
    x2_d = nc.dram_tensor("x2", [N2, D], F32, kind="ExternalInput")
    y_gm = nc.dram_tensor("y_gm", [128, MT], F32, kind="ExternalOutput")
    y_sq1 = nc.dram_tensor("y_sq1", [128, MT], F32, kind="ExternalOutput")

    with tile.TileContext(nc) as tc:
        with tc.tile_pool(name="big", bufs=1) as big, \
             tc.tile_pool(name="src", bufs=12) as srcp, \
             tc.tile_pool(name="work", bufs=2) as workp, \
             tc.tile_pool(name="ps", bufs=2, space="PSUM") as psp:

            X1T = big.tile([128, KT, N1], BF)       # x1 shard, transposed
            X2T = big.tile([128, KT, N2], BF)       # x2 shard, transposed
            X2Tsq = big.tile([128, KT, N2], BF)     # elementwise squares
            gm_out = big.tile([128, MT], F32)
            sq1_out = big.tile([128, MT], F32)
            row0f = big.tile([1, N2], F32)          # -0.5 * ||x2[n]||^2
            row0b = big.tile([128, N2], F32)        # broadcast to all partitions
            ones_w = big.tile([128, 1], BF)

            nc.vector.memset(ones_w[:], 1.0)

            # ---- loads: x1 group 0 first (mains need X1T t0 earliest), then
            #      x2 (4+5 tiles), then rest of x1. One SWDGE queue, in order.
            x1_view = x1_d.ap().rearrange("(g tl p) d -> g p tl d", g=4, tl=4, p=128)
            x1srcs = []
            x1src0 = srcp.tile([128, 4, D], BF, tag="x1src", bufs=4, name="x1src0")
            nc.gpsimd.dma_start(out=x1src0[:], in_=x1_view[0])
            x1srcs.append(x1src0)

            x2a = srcp.tile([128, 4, D], BF, tag="x2a", bufs=1, name="x2a")
            nc.gpsimd.dma_start(
                out=x2a[:],
                in_=x2_d.ap()[0:512, :].rearrange("(tl p) d -> p tl d", p=128))
            x2b = srcp.tile([128, 5, D], BF, tag="x2b", bufs=1, name="x2b")
            nc.gpsimd.dma_start(
                out=x2b[:],
                in_=x2_d.ap()[512:N2, :].rearrange("(tl p) d -> p tl d", p=128))

            for g in range(1, 4):
                x1src = srcp.tile([128, 4, D], BF, tag="x1src", bufs=4,
                                  name=f"x1src{g}")
                nc.gpsimd.dma_start(out=x1src[:], in_=x1_view[g])
                x1srcs.append(x1src)

            # ---- transposes: x1 t0-7 on ACT HWDGE, x2 + x1 t8-15 on SP.
            #      Square each transposed x2 slice immediately (DVE). ----
            for tl in range(4):
                nc.scalar.dma_start(out=X1T[:, :, tl * 128:(tl + 1) * 128],
                                    in_=x1src0[:, tl, :], transpose=True)

            def x2_transpose(t, src, tl):
                nc.sync.dma_start(out=X2T[:, :, t * 128:(t + 1) * 128],
                                  in_=src[:, tl, :], transpose=True)
                nc.vector.tensor_mul(X2Tsq[:, :, t * 128:(t + 1) * 128],
                                     X2T[:, :, t * 128:(t + 1) * 128],
                                     X2T[:, :, t * 128:(t + 1) * 128])

            for tl in range(4):
                x2_transpose(tl, x2a, tl)
            for tl in range(5):
                x2_transpose(4 + tl, x2b, tl)

            for g in range(1, 4):
                for tl in range(4):
                    t = 4 * g + tl
                    dma_eng = nc.scalar if g < 1 else nc.sync
                    dma_eng.dma_start(out=X1T[:, :, t * 128:(t + 1) * 128],
                                      in_=x1srcs[g][:, tl, :], transpose=True)

            # ---- main loop. PE order: m0 mains, sq2 ones-matmul, m1.. mains.
            #      sq2 row is folded in on DVE (no PE appends): the group
            #      statistic is max(cross - 0.5*sq2) over each 9-column group.
            def mains(m):
                ps = psp.tile([128, N2], F32, tag="mm", name=f"ps{m}")
                for k in range(KT):
                    for c, (off, w) in enumerate(CHUNKS):
                        nc.tensor.matmul(ps[:, off:off + w],
                                         lhsT=X1T[:, k, m * 128:(m + 1) * 128],
                                         rhs=X2T[:, k, off:off + w],
                                         start=(k == 0), stop=(k == KT - 1))
                return ps

            def epilogue(m, ps):
                e_bf = workp.tile([128, N2], BF, tag="ebuf", name=f"ebuf{m}")
                nc.vector.tensor_add(e_bf[:], ps[:], row0b[:])
                gmax = workp.tile([128, NG], F32, tag="gmax", name=f"gmax{m}")
                nc.vector.tensor_reduce(
                    out=gmax[:], in_=e_bf[:].rearrange("p (g n) -> p g n", n=GROUP),
                    axis=AX.X, op=mybir.AluOpType.max)
                nc.vector.reduce_sum(out=gm_out[:, m:m + 1], in_=gmax[:], axis=AX.X)

            ps0 = mains(0)

            # sq2 row: column-sum of squares via ones-matmul, then -0.5x and
            # broadcast to all partitions (GPSIMD) for the DVE epilogue add
            ps_row = psp.tile([1, N2], F32, tag="mm")
            for k in range(KT):
                for c, (off, w) in enumerate(CHUNKS):
                    nc.tensor.matmul(ps_row[:, off:off + w], lhsT=ones_w[:],
                                     rhs=X2Tsq[:, k, off:off + w],
                                     start=(k == 0), stop=(k == KT - 1))
            nc.vector.tensor_scalar_mul(row0f[:], ps_row[:], -0.5)
            nc.gpsimd.partition_broadcast(row0b[:], row0f[:])

            epilogue(0, ps0)
            for m in range(1, MT):
                ps = mains(m)
                epilogue(m, ps)

            # ---- sq1 via ACT square-accumulate (feeds only the output) ----
            for g in range(4):
                for tl in range(4):
                    t = 4 * g + tl
                    act_sc = workp.tile([128, D], BF, tag="actsc", name=f"actsc{t}")
                    nc.scalar.activation(out=act_sc[:], in_=x1srcs[g][:, tl, :],
                                         func=AF.Square,
                                         accum_out=sq1_out[:, t:t + 1])

            nc.sync.dma_start(out=y_gm.ap(), in_=gm_out[:])
            nc.sync.dma_start(out=y_sq1.ap(), in_=sq1_out[:])

    nc.compile()
    return nc


def get_nc():
    if "nc" not in _CACHE:
        _CACHE["nc"] = _build()
    return _CACHE["nc"]


def kernel(x1, x2):
    from concourse import bass_utils

    x1 = np.asarray(x1, dtype=np.float32)
    x2 = np.asarray(x2, dtype=np.float32)
    assert x1.shape == (B, N1, D) and x2.shape == (B, N2, D)

    nc = get_nc()
    # shard: batch b -> core b
    in_maps = [{"x1": x1[b], "x2": x2[b]} for b in range(B)]
    res = bass_utils.run_bass_kernel_spmd(nc, in_maps, core_ids=list(range(B)))

    # unshard: combine per-core partial sums (the all-reduce of the mean)
    total = 0.0
    for b in range(B):
        gm = np.asarray(res.results[b]["y_gm"], dtype=np.float64)
        sq1 = np.asarray(res.results[b]["y_sq1"], dtype=np.float64)
        total += NG * sq1.sum() - 2.0 * gm.sum()
    mean = total / (B * N1 * NG)
    return np.asarray(mean, dtype=np.float32)



# revision 3
# speedup vs baseline: 1.1937x; 1.1937x over previous
"""Trainium2 Bass kernel: batched pairwise-distance + group-min-of-9 + mean.

x1 [8, 2048, 1024] f32, x2 [8, 1152, 1024] f32:
    d[b,m,n] = ||x1[b,m] - x2[b,n]||^2
    out = mean over groups-of-9 minima (reshape [B,-1,9].min(-1).mean())

Strategy (data-parallel over B across 8 cores; one batch per core):
  - Host packs x1/x2 to fp8e4; DRAM tensors are uint16 views of fp8
    byte-pairs.
  - xbar transposes of u16 pairs go direct DRAM->SBUF: partition k of
    superchunk c holds fp8 elements d = 256c + 2k + {0,1} -- exactly the
    DoubleRow [K, 2, M] pair layout via bitcast APs.
  - cross matmuls run fp8 DoubleRow (0.5 cyc/col, K=256/pass).
  - sq2 fold: Gram blocks X2T.T@X2T on PE; tensor_tensor_reduce with an
    identity matrix extracts the diagonal pre-scaled to
    delta = -0.5*(sq2 - S_BAR); per-block mini-DMAs gather the columns into
    a [1, N2] row, cast to fp8. A K=1 pair-matmul folds delta into each
    psum window, so the group statistic is max(cross + delta) and
    d_min = sq1 + S_BAR - 2*max.
  - sq1 via ACT Square+accumulate over the fp8 X1T (device-side).
  - group-max-of-9: per m-tile a [0:1008] reduce fires pre-close (overlaps
    that tile's last window on PE) and only a [1008:1152] reduce runs
    post-close, so the 2-slot PSUM rotation never stalls the PE. Reduces
    are split across DVE (tensor_reduce) and GPSIMD/Pool (max trees).
  - ACT sums the per-group maxima into two partials; host combines.
"""
import os
import sys

for _p in ("/opt/trn_rl_repo",):
    if os.path.isdir(_p) and _p not in sys.path:
        sys.path.append(_p)

import numpy as np

B = 8
N1, D, N2 = 2048, 1024, 1152
QD = D // 2                 # 512 u16 columns (fp8 pairs)
GROUP = 9
NG = N2 // GROUP            # 128 groups per row
MT = N1 // 128              # 16 m-tiles
NCH = 4                     # superchunks of K=256
S_BAR = float(D)            # E[||x2||^2]; delta = -0.5*(sq2 - S_BAR)

WINDOWS = ((0, 512), (512, 512), (1024, 128))

# reduce engine assignment per m-tile
PRE_POOL = {1, 4, 7, 10, 13, 14}    # [0:1008] reduce on Pool (else DVE)
TAIL_POOL = {0, 3, 6, 9, 12}        # [1008:1152] reduce on Pool (else DVE)

_CACHE = {}


def _build():
    from concourse import bacc, tile, mybir

    F32 = mybir.dt.float32
    BF16 = mybir.dt.bfloat16
    FP8 = mybir.dt.float8e4
    U16 = mybir.dt.uint16
    AX = mybir.AxisListType
    ALU = mybir.AluOpType
    AF = mybir.ActivationFunctionType
    DR = mybir.MatmulPerfMode.DoubleRow

    nc = bacc.Bacc("TRN2", target_bir_lowering=False, debug=False,
                   num_devices=B, dynamic_dma_scratch_size=65536)
    x1_d = nc.dram_tensor("x1", [N1, QD], U16, kind="ExternalInput")
    x2_d = nc.dram_tensor("x2", [N2, QD], U16, kind="ExternalInput")
    id_d = nc.dram_tensor("ident", [128, 128], BF16, kind="ExternalInput")
    y_gm = nc.dram_tensor("y_gm", [128, 2], F32, kind="ExternalOutput")
    y_sq1 = nc.dram_tensor("y_sq1", [128, 4], F32, kind="ExternalOutput")

    def fp8pair(ap_u16):
        # [128, W] u16 slice -> [128, 2, W] fp8 AP (pair dim outer)
        return ap_u16.bitcast(FP8).rearrange("p (w two) -> p two w", two=2)

    with tile.TileContext(nc) as tc:
        with tc.tile_pool(name="sb", bufs=1) as sb, \
             tc.tile_pool(name="ps", bufs=1, space="PSUM") as psp:
            X1T = sb.tile([128, NCH, N1], U16)
            X2T = sb.tile([128, NCH, N2], U16)
            ident = sb.tile([128, 128], BF16)
            ones8 = sb.tile([1, 2, 128], FP8)
            d8 = sb.tile([1, 2, N2], FP8)       # delta row (pair row 1 = 0)
            drow = sb.tile([1, N2], F32)        # delta row f32 staging
            dcols = sb.tile([128, GROUP], F32)  # per-block diag accums
            ttr_o = sb.tile([128, 128], F32)    # TTR elementwise out scratch
            gmax = sb.tile([128, MT, NG], F32)
            t1 = sb.tile([128, NG, 4], F32)     # pool tree temps
            t2 = sb.tile([128, NG, 2], F32)
            t3 = sb.tile([128, NG], F32)
            dump = sb.tile([128, 4096], BF16)   # ACT mandatory-out scratch
            sq1s = sb.tile([128, 4], F32)
            gsum = sb.tile([128, 2], F32)

            psA = psp.tile([128, N2], F32)      # banks 0-2
            psB = psp.tile([128, N2], F32)      # banks 3-5
            psJ = psp.tile([128, 128], F32)     # bank 6: warmup junk
            psG = psp.tile([128, 128], F32)     # bank 7: Gram blocks

            nc.vector.memset(ones8[:], 0.0)
            nc.vector.memset(ones8[0:1, 0, :], 1.0)
            nc.gpsimd.memset(d8[:], 0.0)
            nc.scalar.dma_start(out=ident[:], in_=id_d.ap())

            # ---- transposes: u16 pairs, direct DRAM -> SBUF ----
            def x2T(q):
                nc.sync.dma_start(
                    out=X2T[:, :, 128 * q:128 * (q + 1)],
                    in_=x2_d.ap()[128 * q:128 * (q + 1), :], transpose=True)

            def x1T(t):
                nc.sync.dma_start(
                    out=X1T[:, :, 128 * t:128 * (t + 1)],
                    in_=x1_d.ap()[128 * t:128 * (t + 1), :], transpose=True)

            for q in range(4):
                x2T(q)
            x1T(0)
            for q in range(4, 9):
                x2T(q)
            for t in range(1, MT):
                x1T(t)

            # ---- PE warmup junk (absorbs the pstate ramp) ----
            def junk(n):
                for _ in range(n):
                    nc.tensor.matmul(
                        psJ[:], lhsT=fp8pair(X2T[:, 0, 0:128]),
                        rhs=fp8pair(X2T[:, 0, 0:128]),
                        start=True, stop=True, perf_mode=DR)

            # ---- Gram-sq2 block g: diag -> delta column -> row gather ----
            def g2block(g):
                blk = X2T[:, :, 128 * g:128 * (g + 1)]
                for c in range(NCH):
                    nc.tensor.matmul(
                        psG[:], lhsT=fp8pair(blk[:, c, :]),
                        rhs=fp8pair(blk[:, c, :]),
                        start=(c == 0), stop=(c == NCH - 1), perf_mode=DR)
                nc.vector.tensor_tensor_reduce(
                    out=ttr_o[:], in0=psG[:], in1=ident[:],
                    scale=-0.5, scalar=0.5 * S_BAR,
                    op0=ALU.mult, op1=ALU.add, accum_out=dcols[:, g:g + 1])
                nc.scalar.dma_start(out=drow[0:1, 128 * g:128 * (g + 1)],
                                    in_=dcols[:, g:g + 1])
                nc.scalar.activation(out=d8[0:1, 0, 128 * g:128 * (g + 1)],
                                     in_=drow[0:1, 128 * g:128 * (g + 1)],
                                     func=AF.Copy)

            junk(10)
            for g in range(9):
                g2block(g)
                junk(8)

            # ---- sq1: ACT Square+accum over fp8 X1T, 4 chunks ----
            def sq1chunk(j):
                seg = X1T[:, :, 512 * j:512 * (j + 1)].bitcast(FP8)
                nc.scalar.activation(
                    out=dump[:].rearrange("p (a b) -> p a b", a=NCH),
                    in_=seg, func=AF.Square, accum_out=sq1s[:, j:j + 1])

            # ---- mains + reduces ----
            def grouped(ps, n0, ngr):
                return ps[:, n0:n0 + 9 * ngr].rearrange(
                    "p (g j) -> p g j", j=GROUP)

            def reduce_dve(ps, n0, ngr, out_ap):
                nc.vector.tensor_reduce(out=out_ap, in_=grouped(ps, n0, ngr),
                                        axis=AX.X, op=ALU.max)

            def reduce_pool(ps, n0, ngr, out_ap):
                gv = grouped(ps, n0, ngr)
                nc.gpsimd.tensor_tensor(out=t1[:, 0:ngr, :], in0=gv[:, :, 0:4],
                                        in1=gv[:, :, 4:8], op=ALU.max)
                nc.gpsimd.tensor_tensor(out=t2[:, 0:ngr, :],
                                        in0=t1[:, 0:ngr, 0:2],
                                        in1=t1[:, 0:ngr, 2:4], op=ALU.max)
                nc.gpsimd.tensor_tensor(out=t3[:, 0:ngr], in0=t2[:, 0:ngr, 0],
                                        in1=t2[:, 0:ngr, 1], op=ALU.max)
                nc.gpsimd.tensor_tensor(out=out_ap, in0=t3[:, 0:ngr],
                                        in1=gv[:, :, 8], op=ALU.max)

            def mains(k):
                ps = psA if k % 2 == 0 else psB
                m0 = 128 * k
                lhsT = [fp8pair(X1T[:, c, m0:m0 + 128]) for c in range(NCH)]
                for wi, (n0, w) in enumerate(WINDOWS):
                    for c in range(NCH):
                        nc.tensor.matmul(
                            ps[:, n0:n0 + w], lhsT=lhsT[c],
                            rhs=fp8pair(X2T[:, c, n0:n0 + w]),
                            start=(c == 0), stop=False, perf_mode=DR)
                    # delta fold closes this window's accumulation group
                    nc.tensor.matmul(
                        ps[:, n0:n0 + w], lhsT=ones8[:],
                        rhs=d8[0:1, :, n0:n0 + w],
                        start=False, stop=True, perf_mode=DR)
                    if wi == 1:
                        # windows 0+1 closed: reduce n[0:1008] while the PE
                        # runs window 2 of the same tile
                        if k in PRE_POOL:
                            reduce_pool(ps, 0, 112, gmax[:, k, 0:112])
                        else:
                            reduce_dve(ps, 0, 112, gmax[:, k, 0:112])
                # post-close: only the last 16 groups gate slot reuse
                if k in TAIL_POOL:
                    reduce_pool(ps, 1008, 16, gmax[:, k, 112:128])
                else:
                    reduce_dve(ps, 1008, 16, gmax[:, k, 112:128])

            for k in range(MT):
                mains(k)
                if k % 4 == 3:
                    sq1chunk(k // 4)

            nc.sync.dma_start(out=y_sq1.ap(), in_=sq1s[:])

            # ---- y_gm: ACT sums of group maxima, two partials ----
            nc.scalar.activation(
                out=dump[:, 0:1792], func=AF.Copy,
                in_=gmax[:, 0:14, :].rearrange("p a b -> p (a b)"),
                accum_out=gsum[:, 0:1])
            nc.scalar.activation(
                out=dump[:, 0:256], func=AF.Copy,
                in_=gmax[:, 14:16, :].rearrange("p a b -> p (a b)"),
                accum_out=gsum[:, 1:2])
            nc.sync.dma_start(out=y_gm.ap(), in_=gsum[:])

    nc.compile()
    return nc


def get_nc():
    if "nc" not in _CACHE:
        _CACHE["nc"] = _build()
    return _CACHE["nc"]


def pack_inputs(x1_b, x2_b):
    import ml_dtypes
    x1q = np.asarray(x1_b, dtype=np.float32).astype(ml_dtypes.float8_e4m3)
    x2q = np.asarray(x2_b, dtype=np.float32).astype(ml_dtypes.float8_e4m3)
    ident = np.eye(128, dtype=ml_dtypes.bfloat16)
    return {"x1": x1q.view(np.uint16), "x2": x2q.view(np.uint16),
            "ident": ident}


def combine(y_gm_arr, y_sq1_arr):
    """Per-core mean: (NG*sum(sq1) + N1*NG*S_BAR - 2*sum(gmax)) / (N1*NG)."""
    gm = np.asarray(y_gm_arr, dtype=np.float64).sum()
    sq1 = np.asarray(y_sq1_arr, dtype=np.float64).sum()
    return (NG * sq1 + N1 * NG * S_BAR - 2.0 * gm) / (N1 * NG)


def kernel(x1, x2):
    from concourse import bass_utils

    x1 = np.asarray(x1, dtype=np.float32)
    x2 = np.asarray(x2, dtype=np.float32)
    assert x1.shape == (B, N1, D) and x2.shape == (B, N2, D)

    nc = get_nc()
    in_maps = [pack_inputs(x1[b], x2[b]) for b in range(B)]
    res = bass_utils.run_bass_kernel_spmd(nc, in_maps, core_ids=list(range(B)))

    mean = np.mean([combine(res.results[b]["y_gm"], res.results[b]["y_sq1"])
                    for b in range(B)])
    return np.asarray(mean, dtype=np.float32)
